# revision 11
# baseline (speedup 1.0000x reference)
"""Conv2d 3x3 (B=32, Cin=128, H=W=56, Cout=256, pad=1, stride=1) + bias.

Strategy: data-parallel over batch across 8 NeuronCores (4 images/core).
Per core, 1D Winograd F(2,3) along the width axis, direct along height:
the 3 horizontal taps (6 madds per 2 outputs) become 4 transformed taps
(4 madds per 2 outputs) -> 1.5x less TensorE work than direct conv.

  V_j[ci, y, tx] = (B^T d)_j with d = xpad[ci, y, 2tx .. 2tx+3]
  M_j[co, y, tx] = sum_ky U_{j,ky}[ci, co]^T @ V_j[ci, y+ky, tx]   (PSUM acc)
  y_even = M_0 + M_1 + M_2 + bias         (VectorE, PSUM -> SBUF fp16)
  y_odd  = M_1 - M_2 - M_3 + bias

U_{j,ky} = sum_kx G[j,kx] W[co, ci, ky, kx] is host-precomputed (fp16),
laid out [ci, cb, j*3+ky, co_l] so each tap's lhsT is a contiguous slice.
Input transform runs on VectorE in fp16 (4 tensor_tensor ops per row band);
output transform is 4 VectorE ops per tile group reading fp32 PSUM directly,
with bias folded in via scalar_tensor_tensor. No edge clipping anywhere:
x is staged zero-padded [128, 58, 58] per image.

PSUM: each (img, cb, rowtile-of-14) group uses 4 one-bank tiles
[128, 14, 28] fp32 (N=392) -> double-buffered across the 8 banks.
Output is written fp16 and upcast to fp32 on the host.
"""

import numpy as np

import concourse.bass as bass
import concourse.mybir as mybir
import concourse.tile as tile
from concourse import bacc
from concourse.bass_utils import run_bass_kernel_spmd

B, C_IN, H, W = 32, 128, 56, 56
C_OUT, KSZ = 256, 3
N_CORES = 8
B_LOC = B // N_CORES  # 4 images per core
CBLKS = C_OUT // 128  # 2
RT = 14  # output rows per tile group
NT = H // RT  # 4 row tiles
NTX = W // 2  # 28 winograd tiles per row
TAPS = 4  # F(2,3) transformed taps
XP = H + 2  # padded rows/cols


def build_nc():
    fp16 = mybir.dt.float16
    fp32 = mybir.dt.float32
    add = mybir.AluOpType.add
    sub = mybir.AluOpType.subtract

    nc = bacc.Bacc(None, target_bir_lowering=False)
    x = nc.dram_tensor("x", [B_LOC, C_IN, H, W], fp16, kind="ExternalInput")
    wt = nc.dram_tensor("wt", [C_IN, CBLKS, TAPS * KSZ, 128], fp16, kind="ExternalInput")
    bias = nc.dram_tensor("bias", [128, CBLKS], fp32, kind="ExternalInput")
    out = nc.dram_tensor("out", [B_LOC, C_OUT, H, W], fp16, kind="ExternalOutput")

    with tile.TileContext(nc) as tc:
        with (
            tc.tile_pool(name="xin", bufs=2) as xpool,
            tc.tile_pool(name="vin", bufs=2) as vpool,
            tc.tile_pool(name="wpool", bufs=1) as wpool,
            tc.tile_pool(name="spool", bufs=4) as spool,
            tc.tile_pool(name="psum", bufs=2, space="PSUM") as psum_pool,
            tc.tile_pool(name="outp", bufs=6) as opool,
        ):
            # weights + bias on the scalar DMA ring (sync ring carries x bands)
            w_sb = wpool.tile([C_IN, CBLKS, TAPS * KSZ, 128], fp16)
            for cb in range(CBLKS):
                nc.scalar.dma_start(w_sb[:, cb], wt[:, cb])
            bias_sb = wpool.tile([128, CBLKS], fp32)
            nc.scalar.dma_start(bias_sb[:], bias[:, :])

            # HAM pre-warm on a memset scratch tile: absorbs the PE IRAM
            # first-fetch stall and starts the HAM ramp while the first
            # x band is still in flight on the DMA ring.
            warm = wpool.tile([C_IN, 256], fp16)
            warm_ps = psum_pool.tile([128, TAPS, 16, 32], fp32, name="psg")
            nc.gpsimd.memset(warm[:].bitcast(mybir.dt.uint16), 0)
            for _ in range(8):
                nc.tensor.matmul(
                    warm_ps[:, 0, 0:8, :], warm[:, :128], warm[:, :256],
                    start=True, stop=True, skip_group_check=True,
                )

            # row bands for x DMA / input transform (xp row indices)
            bands = [(0, 15), (15, 29), (29, 43), (43, XP)]

            for b in range(B_LOC):
                # zero-padded image [128, 58, 58] fp16
                xp = xpool.tile([C_IN, XP, XP], fp16)
                u16 = xp[:].bitcast(mybir.dt.uint16)
                nc.gpsimd.memset(u16[:, 0:1, :], 0)
                nc.gpsimd.memset(u16[:, XP - 1 : XP, :], 0)
                nc.gpsimd.memset(u16[:, :, 0:1], 0)
                nc.gpsimd.memset(u16[:, :, XP - 1 : XP], 0)
                for r0, r1 in bands:
                    d0, d1 = max(r0, 1), min(r1, XP - 1)
                    nc.sync.dma_start(
                        xp[:, d0:d1, 1 : 1 + W], x[b, :, d0 - 1 : d1 - 1, :]
                    )

                # input transform: V[ci, j, yy, tx], one band at a time so
                # GEMMs start as soon as the first band is transformed
                v = vpool.tile([C_IN, TAPS, XP, NTX], fp16)
                for r0, r1 in bands:
                    a0 = xp[:, r0:r1, 0 : 2 * NTX : 2]
                    a1 = xp[:, r0:r1, 1 : 1 + 2 * NTX : 2]
                    a2 = xp[:, r0:r1, 2 : 2 + 2 * NTX : 2]
                    a3 = xp[:, r0:r1, 3 : 2 + 2 * NTX : 2]
                    nc.vector.tensor_tensor(v[:, 0, r0:r1, :], a0, a2, sub)
                    nc.vector.tensor_tensor(v[:, 1, r0:r1, :], a1, a2, add)
                    nc.vector.tensor_tensor(v[:, 2, r0:r1, :], a2, a1, sub)
                    nc.vector.tensor_tensor(v[:, 3, r0:r1, :], a1, a3, sub)

                for cb in range(CBLKS):
                    bvec = bias_sb[:, cb : cb + 1]
                    for t in range(NT):
                        # one 4-bank tile per group: tap j in its own bank
                        # (matmul dst must not cross a 2KB bank boundary)
                        psg = psum_pool.tile([128, TAPS, 16, 32], fp32, name="psg")
                        ps = [psg[:, j, 0:RT, 0:NTX] for j in range(TAPS)]
                        for j in range(TAPS):
                            for ky in range(KSZ):
                                nc.tensor.matmul(
                                    ps[j],
                                    w_sb[:, cb, j * KSZ + ky, :],
                                    v[:, j, t * RT + ky : t * RT + ky + RT, :],
                                    start=(ky == 0),
                                    stop=(ky == KSZ - 1),
                                    skip_group_check=True,
                                )
                        # DVE/ACT ops may read at most ONE PSUM operand each:
                        # evacuate P0 (+bias) on ScalarE and P3 (-bias) on
                        # VectorE first, then combine with one PSUM tap per op.
                        yt = opool.tile([128, RT, W], fp16)
                        t0 = spool.tile([128, RT, NTX], fp32)
                        t3 = spool.tile([128, RT, NTX], fp32)
                        o1 = spool.tile([128, RT, NTX], fp32)
                        o2 = spool.tile([128, RT, NTX], fp32)
                        nc.scalar.activation(
                            t0[:], ps[0], mybir.ActivationFunctionType.Identity,
                            bias=bvec, scale=1.0,
                        )
                        nc.vector.tensor_scalar_sub(t3[:], ps[3], bvec)
                        # y_even = (P0 + bias) + P1 + P2
                        nc.vector.tensor_tensor(o1[:], t0[:], ps[1], add)
                        nc.vector.tensor_tensor(
                            yt[:, :, 0 : 2 * NTX : 2], o1[:], ps[2], add
                        )
                        # y_odd = P1 - (P2 + (P3 - bias))
                        nc.vector.tensor_tensor(o2[:], t3[:], ps[2], add)
                        nc.vector.tensor_tensor(
                            yt[:, :, 1 : 2 * NTX : 2], ps[1], o2[:], sub
                        )
                        nc.scalar.dma_start(
                            out[b, cb * 128 : (cb + 1) * 128, t * RT : (t + 1) * RT, :],
                            yt[:],
                        )
    nc.finalize()
    return nc


def prep_inputs(x, weight, bias):
    # U[j, ky, co, ci] = sum_kx G[j, kx] * w[co, ci, ky, kx]
    G = np.array(
        [[1, 0, 0], [0.5, 0.5, 0.5], [0.5, -0.5, 0.5], [0, 0, 1]], dtype=np.float64
    )
    U = np.einsum("jk,oiyk->jyoi", G, weight.astype(np.float64))  # [4,3,co,ci]
    # -> [ci, cb, j*3+ky, co_l]
    wt = (
        U.reshape(TAPS * KSZ, CBLKS, 128, C_IN)
        .transpose(3, 1, 0, 2)
        .astype(np.float16)
    )
    wt = np.ascontiguousarray(wt)
    bias_r = np.ascontiguousarray(bias.reshape(CBLKS, 128).T, dtype=np.float32)
    x16 = np.ascontiguousarray(x, dtype=np.float16)
    in_maps = []
    for c in range(N_CORES):
        in_maps.append(
            {
                "x": x16[c * B_LOC : (c + 1) * B_LOC],
                "wt": wt,
                "bias": bias_r,
            }
        )
    return in_maps


_NC_CACHE = {}


def run(x, weight, bias, trace=False, nc=None, tmpdir=None):
    if nc is None:
        nc = _NC_CACHE.get("wino")
        if nc is None:
            nc = _NC_CACHE["wino"] = build_nc()
    in_maps = prep_inputs(np.asarray(x), np.asarray(weight), np.asarray(bias))
    res = run_bass_kernel_spmd(
        nc, in_maps, core_ids=list(range(N_CORES)), trace=trace, tmpdir=tmpdir
    )
    out = np.concatenate([r["out"] for r in res.results], axis=0).astype(np.float32)
    return out, res


def kernel(x, weight, bias):
    out, _ = run(x, weight, bias, trace=False)
    return out


if __name__ == "__main__":
    rng = np.random.default_rng(0)
    x = rng.standard_normal((B, C_IN, H, W), dtype=np.float32)
    w = (rng.standard_normal((C_OUT, C_IN, KSZ, KSZ), dtype=np.float32) * 0.05).astype(
        np.float32
    )
    b = rng.standard_normal((C_OUT,), dtype=np.float32)
    out = kernel(x, w, b)
    print(out.shape, out.dtype)


# revision 12
# speedup vs baseline: 1.2741x; 1.2741x over previous
"""Conv2d 3x3 (B=32, Cin=128, H=W=56, Cout=256, pad=1, stride=1) + bias.

Strategy: data-parallel over batch across 8 NeuronCores (4 images/core).
Per core, 1D Winograd F(2,3) along the width axis, direct along height:
the 3 horizontal taps (6 madds per 2 outputs) become 4 transformed taps
(4 madds per 2 outputs) -> 1.5x less TensorE work than direct conv.

  d_k[y, tx] = xpad[y, 2tx+k]             (k = 0..3)
  V_0 = d0-d2  V_1 = d1+d2  V_2 = d2-d1  V_3 = d1-d3      (VectorE, fp16)
  M_j[co, y, tx] = sum_ky U_{j,ky}[ci, co]^T @ V_j[ci, y+ky, tx]  (PSUM acc)
  y_even = M_0 + M_1 + M_2 + bias
  y_odd  = M_1 - M_2 - M_3 + bias

U_{j,ky} = sum_kx G[j,kx] W[co, ci, ky, kx] is host-precomputed (fp16).

Vector-engine ops cost ~(rows x (0.5ns x elems + 25ns)): every on-device
tensor op here is a single flat contiguous run to avoid per-row overhead:
 - host pre-slices x into the four shifted planes d0..d3 (pure strided
   numpy views, zero arithmetic), each [C_IN, 58, 28] with zero-padded
   border rows/cols -> input transform is 4 flat fp16 tensor_tensor ops
   per image and the x DMA is fully contiguous;
 - V is stored flat [128, 4, 58*28] so every matmul rhs is a flat
   contiguous [128, 392] slice, and each PSUM tap is a flat [128, 392]
   slice of a bank-aligned [128, 4, 512] group tile (2 groups in flight);
 - output is written parity-planar [128, 2, 392] fp16 (even plane, odd
   plane); the host interleaves the planes and upcasts to fp32.

Drain per group: ScalarE evacuates a1 = M_1 + bias (the only engine-2
helper; ScalarE also reads PSUM), then VectorE does 4 flat tensor_tensor
ops touching one PSUM operand each (ISA limit). Output DMA rides the
otherwise-idle GpSimd queue; x on sync, weights on scalar.
"""

import numpy as np

import concourse.bass as bass
import concourse.mybir as mybir
import concourse.tile as tile
from concourse import bacc
from concourse.bass_utils import run_bass_kernel_spmd

B, C_IN, H, W = 32, 128, 56, 56
C_OUT, KSZ = 256, 3
N_CORES = 8
B_LOC = B // N_CORES  # 4 images per core
CBLKS = C_OUT // 128  # 2
RT = 14  # output rows per tile group
NT = H // RT  # 4 row tiles
NTX = W // 2  # 28 winograd tile pairs per row
TAPS = 4  # F(2,3) transformed taps
XR = H + 2  # padded rows
FV = XR * NTX  # 1624 flat V/plane elems per tap
FG = RT * NTX  # 392 flat elems per group


def build_nc():
    fp16 = mybir.dt.float16
    fp32 = mybir.dt.float32
    add = mybir.AluOpType.add
    sub = mybir.AluOpType.subtract

    nc = bacc.Bacc(None, target_bir_lowering=False)
    xs = nc.dram_tensor("xs", [B_LOC, TAPS, C_IN, FV], fp16, kind="ExternalInput")
    wt = nc.dram_tensor("wt", [C_IN, CBLKS, TAPS * KSZ, 128], fp16, kind="ExternalInput")
    bias = nc.dram_tensor("bias", [128, CBLKS], fp32, kind="ExternalInput")
    out = nc.dram_tensor("out", [B_LOC, C_OUT, 2, NT * FG], fp16, kind="ExternalOutput")

    with tile.TileContext(nc) as tc:
        with (
            tc.tile_pool(name="xin", bufs=2) as xpool,
            tc.tile_pool(name="vin", bufs=2) as vpool,
            tc.tile_pool(name="wpool", bufs=1) as wpool,
            tc.tile_pool(name="spool", bufs=4) as spool,
            tc.tile_pool(name="psum", bufs=2, space="PSUM") as psum_pool,
            tc.tile_pool(name="outp", bufs=6) as opool,
        ):
            # weights + bias on the scalar DMA ring (sync ring carries x)
            w_sb = wpool.tile([C_IN, CBLKS, TAPS * KSZ, 128], fp16)
            for cb in range(CBLKS):
                nc.scalar.dma_start(w_sb[:, cb], wt[:, cb])
            bias_sb = wpool.tile([128, CBLKS], fp32)
            nc.scalar.dma_start(bias_sb[:], bias[:, :])

            # HAM pre-warm on a memset scratch tile: absorbs the PE IRAM
            # first-fetch stall and starts the HAM ramp while the first
            # x planes are still in flight on the DMA ring.
            warm = wpool.tile([C_IN, 256], fp16)
            warm_ps = psum_pool.tile([128, TAPS, 512], fp32, name="psg")
            nc.gpsimd.memset(warm[:].bitcast(mybir.dt.uint16), 0)
            for _ in range(8):
                nc.tensor.matmul(
                    warm_ps[:, 0, 0:256], warm[:, :128], warm[:, :256],
                    start=True, stop=True, skip_group_check=True,
                )

            # split DMA + transform in row-halves so GEMMs start early
            halves = [(0, 812), (812, FV)]

            for b in range(B_LOC):
                xk = xpool.tile([C_IN, TAPS, FV], fp16)
                for f0, f1 in halves:
                    for k in range(TAPS):
                        nc.sync.dma_start(xk[:, k, f0:f1], xs[b, k, :, f0:f1])
                # input transform: V_j as flat contiguous fp16 ops
                v = vpool.tile([C_IN, TAPS, FV], fp16)
                for f0, f1 in halves:
                    d0 = xk[:, 0, f0:f1]
                    d1 = xk[:, 1, f0:f1]
                    d2 = xk[:, 2, f0:f1]
                    d3 = xk[:, 3, f0:f1]
                    nc.vector.tensor_tensor(v[:, 0, f0:f1], d0, d2, sub)
                    nc.vector.tensor_tensor(v[:, 1, f0:f1], d1, d2, add)
                    nc.vector.tensor_tensor(v[:, 2, f0:f1], d2, d1, sub)
                    nc.vector.tensor_tensor(v[:, 3, f0:f1], d1, d3, sub)

                for cb in range(CBLKS):
                    bvec = bias_sb[:, cb : cb + 1]
                    for t in range(NT):
                        # one 4-bank PSUM tile per group: tap j in bank j
                        psg = psum_pool.tile([128, TAPS, 512], fp32, name="psg")
                        ps = [psg[:, j, 0:FG] for j in range(TAPS)]
                        for j in range(TAPS):
                            for ky in range(KSZ):
                                f0 = (t * RT + ky) * NTX
                                nc.tensor.matmul(
                                    ps[j],
                                    w_sb[:, cb, j * KSZ + ky, :],
                                    v[:, j, f0 : f0 + FG],
                                    start=(ky == 0),
                                    stop=(ky == KSZ - 1),
                                    skip_group_check=True,
                                )
                        # drain: a1 = M1 + bias on ScalarE (PSUM-capable),
                        # then 4 flat VectorE ops, one PSUM operand each
                        yt = opool.tile([128, 2, FG], fp16)
                        a1 = spool.tile([128, FG], fp32)
                        e1 = spool.tile([128, FG], fp32)
                        o2 = spool.tile([128, FG], fp32)
                        nc.scalar.activation(
                            a1[:], ps[1], mybir.ActivationFunctionType.Identity,
                            bias=bvec, scale=1.0,
                        )
                        nc.vector.tensor_tensor(e1[:], a1[:], ps[0], add)
                        nc.vector.tensor_tensor(yt[:, 0, :], e1[:], ps[2], add)
                        nc.vector.tensor_tensor(o2[:], a1[:], ps[3], sub)
                        nc.vector.tensor_tensor(yt[:, 1, :], o2[:], ps[2], sub)
                        nc.gpsimd.dma_start(
                            out[b, cb * 128 : (cb + 1) * 128, :, t * FG : (t + 1) * FG],
                            yt[:],
                        )
    nc.finalize()
    return nc


def prep_inputs(x, weight, bias):
    # U[j, ky, co, ci] = sum_kx G[j, kx] * w[co, ci, ky, kx]
    G = np.array(
        [[1, 0, 0], [0.5, 0.5, 0.5], [0.5, -0.5, 0.5], [0, 0, 1]], dtype=np.float64
    )
    U = np.einsum("jk,oiyk->jyoi", G, weight.astype(np.float64))  # [4,3,co,ci]
    wt = (
        U.reshape(TAPS * KSZ, CBLKS, 128, C_IN)
        .transpose(3, 1, 0, 2)
        .astype(np.float16)
    )
    wt = np.ascontiguousarray(wt)
    bias_r = np.ascontiguousarray(bias.reshape(CBLKS, 128).T, dtype=np.float32)

    # shifted input planes d_k[y, tx] = xpad[y, 2tx+k]: pure strided slicing
    xp = np.zeros((B, C_IN, XR, W + 2), dtype=np.float16)
    xp[:, :, 1 : 1 + H, 1 : 1 + W] = x
    xs = np.empty((B, TAPS, C_IN, XR, NTX), dtype=np.float16)
    for k in range(TAPS):
        xs[:, k] = xp[:, :, :, k : k + 2 * NTX : 2]
    xs = xs.reshape(B, TAPS, C_IN, FV)

    in_maps = []
    for c in range(N_CORES):
        in_maps.append(
            {
                "xs": np.ascontiguousarray(xs[c * B_LOC : (c + 1) * B_LOC]),
                "wt": wt,
                "bias": bias_r,
            }
        )
    return in_maps


def gather_out(res):
    # parity-planar fp16 [B_LOC, C_OUT, 2, NT*FG] -> [B, C_OUT, H, W] fp32
    o = np.concatenate([r["out"] for r in res.results], axis=0)
    o = o.reshape(B, C_OUT, 2, NT, RT, NTX)
    out = np.empty((B, C_OUT, H, W), dtype=np.float32)
    out[:, :, :, 0::2] = o[:, :, 0].reshape(B, C_OUT, H, NTX)
    out[:, :, :, 1::2] = o[:, :, 1].reshape(B, C_OUT, H, NTX)
    return out


_NC_CACHE = {}


def run(x, weight, bias, trace=False, nc=None, tmpdir=None):
    if nc is None:
        nc = _NC_CACHE.get("wino")
        if nc is None:
            nc = _NC_CACHE["wino"] = build_nc()
    in_maps = prep_inputs(np.asarray(x), np.asarray(weight), np.asarray(bias))
    res = run_bass_kernel_spmd(
        nc, in_maps, core_ids=list(range(N_CORES)), trace=trace, tmpdir=tmpdir
    )
    return gather_out(res), res


def kernel(x, weight, bias):
    out, _ = run(x, weight, bias, trace=False)
    return out


if __name__ == "__main__":
    rng = np.random.default_rng(0)
    x = rng.standard_normal((B, C_IN, H, W), dtype=np.float32)
    w = (rng.standard_normal((C_OUT, C_IN, KSZ, KSZ), dtype=np.float32) * 0.05).astype(
        np.float32
    )
    b = rng.standard_normal((C_OUT,), dtype=np.float32)
    out = kernel(x, w, b)
    print(out.shape, out.dtype)
